# revision 15
# baseline (speedup 1.0000x reference)
"""Trainium2 Bass kernel for nn_CLoss_60748017434788.

Loss:  -mean(v) - mean_i( min_j( sum_k |r_ik - f_jk| - v_j ) )
r: [8192,128] f32, f: [8192,128] f32, v: [8192] f32.

Sharding: 2D over 8 cores, 4 real shards x 2 fake shards.  Each core gets
2048 real rows + 4096 fake rows and returns per-row partial mins; the host
min-combines the two fake halves and takes the mean.  All inputs ship as
bf16 packed in a SINGLE 1-D blob per core; anything cheap to precompute on
the host (lhs feature maps, the transposed fake matrix) ships precomputed
so the device pre-phase is just DMA loads.

On-device algorithm (per core): the PE computes a rank-4-per-coordinate
bilinear proxy of the negated selection score S_ij = -(approx d1_ij) + v_j
from bf16 feature maps (contraction 4*128).  DVE max/max_index (fp16
scores) select the top-8 candidates per real row, gpsimd dma_gather
fetches those fake rows (+v) from an on-device gather table, and DVE
recomputes the exact distances and takes the min.  The coupling matrix
NEGC maps lhs features [1, x, x^2, |x|, x|x|, sign(x), x^3] of r to rhs
features [y, y^2, |y|, y|y|] of f; row k=127 of rhs feature 1 is
sacrificed to carry +v_j (its lhs partner is 1).
"""

import os
import tempfile
import time

import numpy as np
import ml_dtypes

import jax

try:
    jax.config.update(
        "jax_compilation_cache_dir",
        os.path.join(tempfile.gettempdir(), "jax_cache_closs"),
    )
    jax.config.update("jax_persistent_cache_min_entry_size_bytes", -1)
    jax.config.update("jax_persistent_cache_min_compile_time_secs", 0.0)
except Exception:
    pass

NR, NF, D = 8192, 8192, 128
NCORES = 8
RSH, FSH = 4, 2                 # real shards x fake shards
RROWS = NR // RSH               # 2048 real rows per core
FROWS = NF // FSH               # 4096 fake rows per core
NIT = RROWS // 128              # 16 i-tiles per core
NFT = FROWS // 128              # 32 fake 128-tiles per core
JT = 512                        # matmul free-dim tile
NJT = FROWS // JT               # 8 j-tiles
NCAND = 4                       # exact-recompute candidates per row
AUGW = 256                      # bf16 elems per gather row (512B): [f(128), v, pad]
NFEAT = 4

OFF_R, LEN_R = 0, RROWS * D                       # rS   [2048,128] row-major
OFF_F, LEN_F = OFF_R + LEN_R, FROWS * D           # fS   [4096,128] row-major
OFF_FT, LEN_FT = OFF_F + LEN_F, D * FROWS         # fT   [128,4096] row-major
OFF_LF, LEN_LF = OFF_FT + LEN_FT, NFEAT * D * RROWS   # lf[m] [128,2048] each
OFF_V, LEN_V = OFF_LF + LEN_LF, FROWS             # v    [4096]
BLOB = OFF_V + LEN_V

# rows: [1, x, x2, |x|, x|x|, sign, x3] ; cols: rhs [y, y2, |y|, y|y|]
NEGC = np.array([
    [-2.64634495e-03, 2.57689506e-02, -1.16234565e+00, 2.03689490e-03],
    [2.17274690e+00, -1.19240610e-02, 2.07460839e-02, -7.70343959e-01],
    [-5.45617985e-03, 1.79038107e-01, -4.85291958e-01, 3.84314870e-03],
    [9.64919943e-03, -4.85617042e-01, 1.75258219e+00, -6.89594261e-03],
    [-1.13944638e+00, 1.23156002e-02, -2.10905615e-02, 5.43146372e-01],
    [-3.23009975e-02, 1.92518265e-03, -3.08780512e-03, 9.46847629e-03],
    [1.74482226e-01, -3.03717307e-03, 5.07844985e-03, -9.47937220e-02],
], dtype=np.float32)

_CACHE = {}


def build_nc():
    from contextlib import ExitStack

    import concourse.bass as bass  # noqa: F401
    import concourse.mybir as mybir
    import concourse.tile as tile
    from concourse import bacc, library_config
    from concourse.bass import ts

    dt = mybir.dt
    AX = mybir.AxisListType
    OP = mybir.AluOpType
    AF = mybir.ActivationFunctionType

    nc = bacc.Bacc("TRN2", debug=False)
    blob = nc.dram_tensor("blob", [BLOB], dt.bfloat16, kind="ExternalInput")
    outm = nc.dram_tensor("outm", [RROWS], dt.float32, kind="ExternalOutput")

    r_ap = blob.ap()[OFF_R:OFF_R + LEN_R].rearrange(
        "(t p d) -> p t d", p=128, d=D)                      # [128, NIT, D]
    f_ap = blob.ap()[OFF_F:OFF_F + LEN_F].rearrange(
        "(t p d) -> p t d", p=128, d=D)                      # [128, NFT, D]
    ft_ap = blob.ap()[OFF_FT:OFF_FT + LEN_FT].rearrange(
        "(p c) -> p c", p=128)                               # [128, FROWS]
    lf_aps = [blob.ap()[OFF_LF + m * D * RROWS:
                        OFF_LF + (m + 1) * D * RROWS].rearrange(
        "(p c) -> p c", p=128) for m in range(NFEAT)]        # [128, RROWS]
    v_row_ap = blob.ap()[OFF_V:OFF_V + LEN_V][None, :]       # [1, FROWS]
    v_wrap_ap = blob.ap()[OFF_V:OFF_V + LEN_V].rearrange(
        "(t p) -> p t", p=128)                               # [128, NFT]

    with ExitStack() as ctx:
        tc = ctx.enter_context(tile.TileContext(nc))
        persist = ctx.enter_context(tc.tile_pool(name="persist", bufs=1))

        feats = [persist.tile([128, FROWS], dt.bfloat16, tag=f"feat{m}",
                              name=f"feat{m}")
                 for m in range(NFEAT)]
        lf = [persist.tile([128, RROWS], dt.bfloat16, tag=f"lf{m}",
                           name=f"lf{m}")
              for m in range(NFEAT)]
        rt_all = persist.tile([128, NIT, D], dt.bfloat16, tag="rt_all")
        mins_all = persist.tile([128, NIT], dt.float32, tag="mins")
        faug = ctx.enter_context(
            tc.tile_pool(name="dramp", bufs=1, space="DRAM")
        ).tile([FROWS, AUGW], dt.bfloat16, tag="faug", name="faug")

        # ---------------- stage A: loads + rhs features ----------------
        with tc.tile_pool(name="stage", bufs=1) as stage:
            # feats[0] = y loaded directly; lhs features precomputed on host
            nc.sync.dma_start(feats[0][:], ft_ap)
            for m in range(NFEAT):
                nc.sync.dma_start(lf[m][:], lf_aps[m])
            nc.sync.dma_start(rt_all[:], r_ap)
            # rhs features in bf16, chunked so matmuls can start early
            CH = FROWS // 2
            for c0 in (0, CH):
                sl = slice(c0, c0 + CH)
                nc.scalar.activation(feats[1][:, sl], feats[0][:, sl],
                                     AF.Square)
                nc.scalar.activation(feats[2][:, sl], feats[0][:, sl], AF.Abs)
                nc.vector.tensor_tensor(feats[3][:, sl], feats[0][:, sl],
                                        feats[2][:, sl], OP.mult)
            # sacrifice row: rhs feature 1, k=127 carries +v
            nc.sync.dma_start(feats[1][127:128, :], v_row_ap)

            # gather table: fake rows + v appended, written once to DRAM
            fsb = stage.tile([128, NFT, D], dt.bfloat16, tag="fsb")
            nc.scalar.dma_start(fsb[:], f_ap)
            fsa = stage.tile([128, NFT, AUGW], dt.bfloat16, tag="fsa")
            nc.vector.tensor_copy(fsa[:, :, 0:D], fsb[:])
            vsb = stage.tile([128, NFT], dt.bfloat16, tag="vsb")
            nc.scalar.dma_start(vsb[:], v_wrap_ap)
            nc.vector.tensor_copy(fsa[:, :, D], vsb[:])
            nc.scalar.dma_start(
                faug[:].rearrange("(t p) w -> p t w", p=128), fsa[:])

        # ---------------- stage B: proxy + select + exact ----------------
        nc.gpsimd.load_library(library_config.mlp)
        with tc.tile_pool(name="work", bufs=3) as work, \
             tc.tile_pool(name="psum", bufs=8, space="PSUM") as psum, \
             tc.tile_pool(name="drams", bufs=4, space="DRAM") as dpool, \
             tc.tile_pool(name="small", bufs=6) as small:
            def exact(te, fge):
                rt = rt_all[:, te, :]
                diff = work.tile([128, NCAND, D], dt.bfloat16, tag="diff",
                                 name=f"diff{te}")
                nc.vector.tensor_tensor(
                    diff[:], fge[:, :, 0:D],
                    rt[:, None, :].to_broadcast((128, NCAND, D)), OP.subtract)
                d1c = small.tile([128, NCAND], dt.float32, tag="d1c",
                                 name=f"d1c{te}")
                nc.vector.tensor_reduce(d1c[:], diff[:], axis=AX.X, op=OP.add,
                                        apply_absolute_value=True)
                vc = small.tile([128, NCAND], dt.float32, tag="vc",
                                name=f"vc{te}")
                nc.vector.tensor_copy(vc[:], fge[:, :, D])
                gc = small.tile([128, NCAND], dt.float32, tag="gc",
                                name=f"gc{te}")
                nc.vector.tensor_tensor(gc[:], d1c[:], vc[:], OP.subtract)
                nc.vector.tensor_reduce(mins_all[:, te:te + 1], gc[:],
                                        axis=AX.X, op=OP.min)

            pend = []
            for t in range(NIT):
                score = work.tile([128, FROWS], dt.float16, tag="score")
                pss = [psum.tile([128, JT], dt.float32, tag="ps",
                                 name=f"ps{t}_{k}") for k in range(NJT)]
                # two 4-bank sub-groups: copies of group A overlap matmuls
                # of group B so the PE never waits on a full-tile drain
                for jg in range(2):
                    for jj in range(jg * 4, jg * 4 + 4):
                        for m in range(NFEAT):
                            nc.tensor.matmul(
                                pss[jj][:],
                                lf[m][:, ts(t, 128)],
                                feats[m][:, ts(jj, JT)],
                                start=(m == 0), stop=(m == NFEAT - 1))
                    for jj in range(jg * 4, jg * 4 + 4):
                        nc.scalar.copy(score[:, ts(jj, JT)], pss[jj][:])

                mx = small.tile([128, 8], dt.float16, tag="mx")
                nc.vector.max(mx[:], score[:])
                idx = small.tile([128, 8], dt.uint16, tag="idx")
                nc.vector.max_index(idx[:], mx[:], score[:])

                # reshuffle indices to the wrapped dma_gather layout via
                # DRAM; only the top-NCAND of the 8 found indices are used.
                # These stay on the sync queue: on gpsimd they would
                # serialize with the gathers, on scalar with the copies.
                idram = dpool.tile([128 * NCAND], dt.uint16, tag="idram")
                nc.sync.dma_start(idram.rearrange("(p c) -> p c", c=NCAND),
                                  idx[:, 0:NCAND])
                idxw = small.tile([128, 8 * NCAND], dt.uint16, tag="idxw")
                wrap = idram.rearrange("(u tt c) -> tt c u", u=8, tt=16,
                                       c=NCAND)
                for q in range(8):
                    nc.sync.dma_start(
                        idxw[16 * q:16 * (q + 1), :].rearrange(
                            "p (c u) -> p c u", c=NCAND),
                        wrap)

                fg = work.tile([128, NCAND, AUGW], dt.bfloat16, tag="fg")
                nc.gpsimd.dma_gather(
                    fg[:], faug[:], idxw[:].bitcast(dt.int16),
                    num_idxs=NCAND * 128, num_idxs_reg=NCAND * 128,
                    elem_size=AUGW)

                pend.append((t, fg))
                # exact recompute runs two tiles behind selection so the
                # in-order DVE stream never waits on an in-flight gather
                if len(pend) >= 3:
                    exact(*pend.pop(0))

            while pend:
                exact(*pend.pop(0))
            nc.sync.dma_start(outm.ap().rearrange("(t p) -> p t", p=128),
                              mins_all[:])
    nc.compile()
    return nc


def prepare_in_maps(real, fake, v):
    bf = ml_dtypes.bfloat16
    real = np.asarray(real, dtype=np.float32)
    fake = np.asarray(fake, dtype=np.float32)
    v32 = np.asarray(v, dtype=np.float32)
    real_bf = real.astype(bf)
    fake_bf = fake.astype(bf)
    v_bf = v32.astype(bf)

    # lhs feature maps, mixed by NEGC on host in f32:  LF[m] = [NR, D]
    x = real
    ax = np.abs(x)
    basis = np.stack([np.ones_like(x), x, x * x, ax, x * ax, np.sign(x),
                      x * x * x])                       # [7, NR, D]
    LF = np.tensordot(NEGC, basis, axes=(0, 0))         # [4, NR, D] f32
    LF[1, :, 127] = 1.0                                 # sacrifice-row partner
    LFT = LF.astype(bf).transpose(0, 2, 1)              # [4, D, NR]

    in_maps = []
    for c in range(NCORES):
        a, b = c // FSH, c % FSH
        rsl = slice(a * RROWS, (a + 1) * RROWS)
        fsl = slice(b * FROWS, (b + 1) * FROWS)
        blobv = np.empty(BLOB, dtype=bf)
        blobv[OFF_R:OFF_R + LEN_R] = real_bf[rsl].ravel()
        blobv[OFF_F:OFF_F + LEN_F] = fake_bf[fsl].ravel()
        blobv[OFF_FT:OFF_FT + LEN_FT] = \
            np.ascontiguousarray(fake_bf[fsl].T).ravel()
        blobv[OFF_LF:OFF_LF + LEN_LF] = \
            np.ascontiguousarray(LFT[:, :, rsl]).ravel()
        blobv[OFF_V:OFF_V + LEN_V] = v_bf[fsl]
        in_maps.append({"blob": blobv})
    return in_maps


def run(real, fake, v, trace=False):
    from concourse.bass_utils import run_bass_kernel_spmd
    if "nc" not in _CACHE:
        _CACHE["nc"] = build_nc()
    nc = _CACHE["nc"]
    in_maps = prepare_in_maps(real, fake, v)
    try:
        res = run_bass_kernel_spmd(nc, in_maps, core_ids=list(range(NCORES)),
                                   trace=trace)
    except ModuleNotFoundError:
        res = run_bass_kernel_spmd(nc, in_maps, core_ids=list(range(NCORES)),
                                   trace=False)
    except Exception:
        # transient device hiccup (e.g. NRT exec-unit recovery): retry once
        time.sleep(10)
        res = run_bass_kernel_spmd(nc, in_maps, core_ids=list(range(NCORES)),
                                   trace=False)
    mins = np.stack([res.results[c]["outm"] for c in range(NCORES)])
    rowmins = np.minimum(mins[0::FSH], mins[1::FSH])     # [RSH, RROWS]
    vmean = float(np.asarray(v, dtype=np.float32).mean())
    out = np.float32(-vmean - rowmins.mean(dtype=np.float64))
    return out, res


def kernel(real_objects, fake_objects, fake_validity):
    out, _ = run(real_objects, fake_objects, fake_validity)
    return out


# revision 16
# speedup vs baseline: 1.1093x; 1.1093x over previous
"""Trainium2 Bass kernel for nn_CLoss_60748017434788.

Loss:  -mean(v) - mean_i( min_j( sum_k |r_ik - f_jk| - v_j ) )
r: [8192,128] f32, f: [8192,128] f32, v: [8192] f32.

Sharding: 2D over 8 cores, 4 real shards x 2 fake shards.  Each core gets
2048 real rows + 4096 fake rows and returns per-row partial mins; the host
min-combines the two fake halves and takes the mean.  All inputs ship as
bf16 packed in a SINGLE 1-D blob per core; anything cheap to precompute on
the host (lhs feature maps, the transposed fake matrix) ships precomputed
so the device pre-phase is just DMA loads.

On-device algorithm (per core): the PE computes a rank-4-per-coordinate
bilinear proxy of the negated selection score S_ij = -(approx d1_ij) + v_j
from bf16 feature maps (contraction 4*128).  DVE max/max_index (fp16
scores) select the top-8 candidates per real row, gpsimd dma_gather
fetches those fake rows (+v) from an on-device gather table, and DVE
recomputes the exact distances and takes the min.  The coupling matrix
NEGC maps lhs features [1, x, x^2, |x|, x|x|, sign(x), x^3] of r to rhs
features [y, y^2, |y|, y|y|] of f; row k=127 of rhs feature 1 is
sacrificed to carry +v_j (its lhs partner is 1).
"""

import os
import tempfile
import time

import numpy as np
import ml_dtypes

import jax

try:
    jax.config.update(
        "jax_compilation_cache_dir",
        os.path.join(tempfile.gettempdir(), "jax_cache_closs"),
    )
    jax.config.update("jax_persistent_cache_min_entry_size_bytes", -1)
    jax.config.update("jax_persistent_cache_min_compile_time_secs", 0.0)
except Exception:
    pass

NR, NF, D = 8192, 8192, 128
NCORES = 8
RSH, FSH = 4, 2                 # real shards x fake shards
RROWS = NR // RSH               # 2048 real rows per core
FROWS = NF // FSH               # 4096 fake rows per core
NIT = RROWS // 128              # 16 i-tiles per core
NFT = FROWS // 128              # 32 fake 128-tiles per core
JT = 512                        # matmul free-dim tile
NJT = FROWS // JT               # 8 j-tiles
NCAND = 4                       # exact-recompute candidates per row
AUGW = 256                      # bf16 elems per gather row (512B): [f(128), v, pad]
NFEAT = 4

OFF_R, LEN_R = 0, RROWS * D                       # rS   [2048,128] row-major
OFF_F, LEN_F = OFF_R + LEN_R, FROWS * D           # fS   [4096,128] row-major
OFF_V, LEN_V = OFF_F + LEN_F, FROWS               # v    [4096]
BLOB = OFF_V + LEN_V
O8_LF, L8_LF = 0, NFEAT * D * RROWS               # lf[m]    [128,2048] each, f8
O8_FE, L8_FE = L8_LF, NFEAT * D * FROWS           # feats[m] [128,4096] each, f8
BLOB8 = O8_FE + L8_FE

# rows: [1, x, x2, |x|, x|x|, sign, x3] ; cols: rhs [y, y2, |y|, y|y|]
NEGC = np.array([
    [-2.64634495e-03, 2.57689506e-02, -1.16234565e+00, 2.03689490e-03],
    [2.17274690e+00, -1.19240610e-02, 2.07460839e-02, -7.70343959e-01],
    [-5.45617985e-03, 1.79038107e-01, -4.85291958e-01, 3.84314870e-03],
    [9.64919943e-03, -4.85617042e-01, 1.75258219e+00, -6.89594261e-03],
    [-1.13944638e+00, 1.23156002e-02, -2.10905615e-02, 5.43146372e-01],
    [-3.23009975e-02, 1.92518265e-03, -3.08780512e-03, 9.46847629e-03],
    [1.74482226e-01, -3.03717307e-03, 5.07844985e-03, -9.47937220e-02],
], dtype=np.float32)

_CACHE = {}


def build_nc():
    from contextlib import ExitStack

    import concourse.bass as bass  # noqa: F401
    import concourse.mybir as mybir
    import concourse.tile as tile
    from concourse import bacc, library_config
    from concourse.bass import ts

    dt = mybir.dt
    AX = mybir.AxisListType
    OP = mybir.AluOpType
    AF = mybir.ActivationFunctionType

    nc = bacc.Bacc("TRN2", debug=False)
    blob = nc.dram_tensor("blob", [BLOB], dt.bfloat16, kind="ExternalInput")
    blob8 = nc.dram_tensor("blob8", [BLOB8], dt.float8e4,
                           kind="ExternalInput")
    outm = nc.dram_tensor("outm", [RROWS], dt.float32, kind="ExternalOutput")

    r_ap = blob.ap()[OFF_R:OFF_R + LEN_R].rearrange(
        "(t p d) -> p t d", p=128, d=D)                      # [128, NIT, D]
    f_ap = blob.ap()[OFF_F:OFF_F + LEN_F].rearrange(
        "(t p d) -> p t d", p=128, d=D)                      # [128, NFT, D]
    lf_aps = [blob8.ap()[O8_LF + m * D * RROWS:
                         O8_LF + (m + 1) * D * RROWS].rearrange(
        "(p c) -> p c", p=128) for m in range(NFEAT)]        # [128, RROWS]
    fe_aps = [blob8.ap()[O8_FE + m * D * FROWS:
                         O8_FE + (m + 1) * D * FROWS].rearrange(
        "(p c) -> p c", p=128) for m in range(NFEAT)]        # [128, FROWS]
    v_wrap_ap = blob.ap()[OFF_V:OFF_V + LEN_V].rearrange(
        "(t p) -> p t", p=128)                               # [128, NFT]

    with ExitStack() as ctx:
        tc = ctx.enter_context(tile.TileContext(nc))
        persist = ctx.enter_context(tc.tile_pool(name="persist", bufs=1))

        feats = [persist.tile([128, FROWS], dt.float8e4, tag=f"feat{m}",
                              name=f"feat{m}")
                 for m in range(NFEAT)]
        lf = [persist.tile([128, RROWS], dt.float8e4, tag=f"lf{m}",
                           name=f"lf{m}")
              for m in range(NFEAT)]
        rt_all = persist.tile([128, NIT, D], dt.bfloat16, tag="rt_all")
        mins_all = persist.tile([128, NIT], dt.float32, tag="mins")
        faug = ctx.enter_context(
            tc.tile_pool(name="dramp", bufs=1, space="DRAM")
        ).tile([FROWS, AUGW], dt.bfloat16, tag="faug", name="faug")

        # ---------------- stage A: loads (all features host-built) -----
        with tc.tile_pool(name="stage", bufs=1) as stage:
            for m in range(NFEAT):
                nc.sync.dma_start(feats[m][:], fe_aps[m])
                nc.sync.dma_start(lf[m][:], lf_aps[m])
            nc.sync.dma_start(rt_all[:], r_ap)

            # gather table: fake rows + v appended, written once to DRAM
            fsb = stage.tile([128, NFT, D], dt.bfloat16, tag="fsb")
            nc.scalar.dma_start(fsb[:], f_ap)
            fsa = stage.tile([128, NFT, AUGW], dt.bfloat16, tag="fsa")
            nc.vector.tensor_copy(fsa[:, :, 0:D], fsb[:])
            vsb = stage.tile([128, NFT], dt.bfloat16, tag="vsb")
            nc.scalar.dma_start(vsb[:], v_wrap_ap)
            nc.vector.tensor_copy(fsa[:, :, D], vsb[:])
            nc.scalar.dma_start(
                faug[:].rearrange("(t p) w -> p t w", p=128), fsa[:])

        # ---------------- stage B: proxy + select + exact ----------------
        nc.gpsimd.load_library(library_config.mlp)
        with tc.tile_pool(name="work", bufs=3) as work, \
             tc.tile_pool(name="psum", bufs=8, space="PSUM") as psum, \
             tc.tile_pool(name="drams", bufs=4, space="DRAM") as dpool, \
             tc.tile_pool(name="small", bufs=6) as small:
            def exact(te, fge):
                rt = rt_all[:, te, :]
                diff = work.tile([128, NCAND, D], dt.bfloat16, tag="diff",
                                 name=f"diff{te}")
                nc.vector.tensor_tensor(
                    diff[:], fge[:, :, 0:D],
                    rt[:, None, :].to_broadcast((128, NCAND, D)), OP.subtract)
                d1c = small.tile([128, NCAND], dt.float32, tag="d1c",
                                 name=f"d1c{te}")
                nc.vector.tensor_reduce(d1c[:], diff[:], axis=AX.X, op=OP.add,
                                        apply_absolute_value=True)
                vc = small.tile([128, NCAND], dt.float32, tag="vc",
                                name=f"vc{te}")
                nc.vector.tensor_copy(vc[:], fge[:, :, D])
                gc = small.tile([128, NCAND], dt.float32, tag="gc",
                                name=f"gc{te}")
                nc.vector.tensor_tensor(gc[:], d1c[:], vc[:], OP.subtract)
                nc.vector.tensor_reduce(mins_all[:, te:te + 1], gc[:],
                                        axis=AX.X, op=OP.min)

            pend = []
            for t in range(NIT):
                score = work.tile([128, FROWS], dt.float16, tag="score")
                pss = [psum.tile([128, JT], dt.float32, tag="ps",
                                 name=f"ps{t}_{k}") for k in range(NJT)]
                # two 4-bank sub-groups: copies of group A overlap matmuls
                # of group B so the PE never waits on a full-tile drain
                for jg in range(2):
                    for jj in range(jg * 4, jg * 4 + 4):
                        for m in range(NFEAT):
                            nc.tensor.matmul(
                                pss[jj][:],
                                lf[m][:, ts(t, 128)],
                                feats[m][:, ts(jj, JT)],
                                start=(m == 0), stop=(m == NFEAT - 1))
                    for jj in range(jg * 4, jg * 4 + 4):
                        nc.scalar.copy(score[:, ts(jj, JT)], pss[jj][:])

                mx = small.tile([128, 8], dt.float16, tag="mx")
                nc.vector.max(mx[:], score[:])
                idx = small.tile([128, 8], dt.uint16, tag="idx")
                nc.vector.max_index(idx[:], mx[:], score[:])

                # reshuffle indices to the wrapped dma_gather layout via
                # DRAM; only the top-NCAND of the 8 found indices are used.
                # These stay on the sync queue: on gpsimd they would
                # serialize with the gathers, on scalar with the copies.
                idram = dpool.tile([128 * NCAND], dt.uint16, tag="idram")
                nc.sync.dma_start(idram.rearrange("(p c) -> p c", c=NCAND),
                                  idx[:, 0:NCAND])
                idxw = small.tile([128, 8 * NCAND], dt.uint16, tag="idxw")
                wrap = idram.rearrange("(u tt c) -> tt c u", u=8, tt=16,
                                       c=NCAND)
                for q in range(8):
                    nc.sync.dma_start(
                        idxw[16 * q:16 * (q + 1), :].rearrange(
                            "p (c u) -> p c u", c=NCAND),
                        wrap)

                fg = work.tile([128, NCAND, AUGW], dt.bfloat16, tag="fg")
                nc.gpsimd.dma_gather(
                    fg[:], faug[:], idxw[:].bitcast(dt.int16),
                    num_idxs=NCAND * 128, num_idxs_reg=NCAND * 128,
                    elem_size=AUGW)

                pend.append((t, fg))
                # exact recompute runs two tiles behind selection so the
                # in-order DVE stream never waits on an in-flight gather
                if len(pend) >= 3:
                    exact(*pend.pop(0))

            while pend:
                exact(*pend.pop(0))
            nc.sync.dma_start(outm.ap().rearrange("(t p) -> p t", p=128),
                              mins_all[:])
    nc.compile()
    return nc


def prepare_in_maps(real, fake, v):
    bf = ml_dtypes.bfloat16
    real = np.asarray(real, dtype=np.float32)
    fake = np.asarray(fake, dtype=np.float32)
    v32 = np.asarray(v, dtype=np.float32)
    real_bf = real.astype(bf)
    fake_bf = fake.astype(bf)
    v_bf = v32.astype(bf)

    f8 = ml_dtypes.float8_e4m3
    # lhs feature maps, mixed by NEGC on host in f32:  LF[m] = [NR, D]
    x = real
    ax = np.abs(x)
    basis = np.stack([np.ones_like(x), x, x * x, ax, x * ax, np.sign(x),
                      x * x * x])                       # [7, NR, D]
    LF = np.tensordot(NEGC, basis, axes=(0, 0))         # [4, NR, D] f32
    LF[1, :, 127] = 1.0                                 # sacrifice-row partner
    LFT = LF.astype(f8).transpose(0, 2, 1)              # [4, D, NR]
    # rhs feature maps [y, y^2, |y|, y|y|]; feature 1 row k=127 carries +v
    y = fake
    ay = np.abs(y)
    FE = np.stack([y, y * y, ay, y * ay])               # [4, NF, D] f32
    FE[1, :, 127] = v32
    FET = FE.astype(f8).transpose(0, 2, 1)              # [4, D, NF]

    in_maps = []
    for c in range(NCORES):
        a, b = c // FSH, c % FSH
        rsl = slice(a * RROWS, (a + 1) * RROWS)
        fsl = slice(b * FROWS, (b + 1) * FROWS)
        blobv = np.empty(BLOB, dtype=bf)
        blobv[OFF_R:OFF_R + LEN_R] = real_bf[rsl].ravel()
        blobv[OFF_F:OFF_F + LEN_F] = fake_bf[fsl].ravel()
        blobv[OFF_V:OFF_V + LEN_V] = v_bf[fsl]
        blob8v = np.empty(BLOB8, dtype=f8)
        blob8v[O8_LF:O8_LF + L8_LF] = \
            np.ascontiguousarray(LFT[:, :, rsl]).ravel()
        blob8v[O8_FE:O8_FE + L8_FE] = \
            np.ascontiguousarray(FET[:, :, fsl]).ravel()
        in_maps.append({"blob": blobv, "blob8": blob8v})
    return in_maps


def run(real, fake, v, trace=False):
    from concourse.bass_utils import run_bass_kernel_spmd
    if "nc" not in _CACHE:
        _CACHE["nc"] = build_nc()
    nc = _CACHE["nc"]
    in_maps = prepare_in_maps(real, fake, v)
    try:
        res = run_bass_kernel_spmd(nc, in_maps, core_ids=list(range(NCORES)),
                                   trace=trace)
    except ModuleNotFoundError:
        res = run_bass_kernel_spmd(nc, in_maps, core_ids=list(range(NCORES)),
                                   trace=False)
    except Exception:
        # transient device hiccup (e.g. NRT exec-unit recovery): retry once
        time.sleep(10)
        res = run_bass_kernel_spmd(nc, in_maps, core_ids=list(range(NCORES)),
                                   trace=False)
    mins = np.stack([res.results[c]["outm"] for c in range(NCORES)])
    rowmins = np.minimum(mins[0::FSH], mins[1::FSH])     # [RSH, RROWS]
    vmean = float(np.asarray(v, dtype=np.float32).mean())
    out = np.float32(-vmean - rowmins.mean(dtype=np.float64))
    return out, res


def kernel(real_objects, fake_objects, fake_validity):
    out, _ = run(real_objects, fake_objects, fake_validity)
    return out


# revision 17
# speedup vs baseline: 1.2063x; 1.0875x over previous
"""Trainium2 Bass kernel for nn_CLoss_60748017434788.

Loss:  -mean(v) - mean_i( min_j( sum_k |r_ik - f_jk| - v_j ) )
r: [8192,128] f32, f: [8192,128] f32, v: [8192] f32.

Sharding: 2D over 8 cores, 4 real shards x 2 fake shards.  Each core gets
2048 real rows + 4096 fake rows and returns per-row partial mins; the host
min-combines the two fake halves and takes the mean.  All inputs ship as
bf16 packed in a SINGLE 1-D blob per core; anything cheap to precompute on
the host (lhs feature maps, the transposed fake matrix) ships precomputed
so the device pre-phase is just DMA loads.

On-device algorithm (per core): the PE computes a rank-4-per-coordinate
bilinear proxy of the negated selection score S_ij = -(approx d1_ij) + v_j
from bf16 feature maps (contraction 4*128).  DVE max/max_index (fp16
scores) select the top-8 candidates per real row, gpsimd dma_gather
fetches those fake rows (+v) from an on-device gather table, and DVE
recomputes the exact distances and takes the min.  The coupling matrix
NEGC maps lhs features [1, x, x^2, |x|, x|x|, sign(x), x^3] of r to rhs
features [y, y^2, |y|, y|y|] of f; row k=127 of rhs feature 1 is
sacrificed to carry +v_j (its lhs partner is 1).
"""

import os
import tempfile
import time

import numpy as np
import ml_dtypes

import jax

try:
    jax.config.update(
        "jax_compilation_cache_dir",
        os.path.join(tempfile.gettempdir(), "jax_cache_closs"),
    )
    jax.config.update("jax_persistent_cache_min_entry_size_bytes", -1)
    jax.config.update("jax_persistent_cache_min_compile_time_secs", 0.0)
except Exception:
    pass

NR, NF, D = 8192, 8192, 128
NCORES = 8
RSH, FSH = 4, 2                 # real shards x fake shards
RROWS = NR // RSH               # 2048 real rows per core
FROWS = NF // FSH               # 4096 fake rows per core
NIT = RROWS // 128              # 16 i-tiles per core
NFT = FROWS // 128              # 32 fake 128-tiles per core
JT = 512                        # matmul free-dim tile
NJT = FROWS // JT               # 8 j-tiles
NCAND = 4                       # exact-recompute candidates per row
AUGW = 256                      # bf16 elems per gather row (512B): [f(128), v, pad]
NFEAT = 4

OFF_R, LEN_R = 0, RROWS * D                       # rS   [2048,128] row-major
OFF_F, LEN_F = OFF_R + LEN_R, FROWS * D           # fS   [4096,128] row-major
OFF_V, LEN_V = OFF_F + LEN_F, FROWS               # v    [4096]
BLOB = OFF_V + LEN_V
O8_LF, L8_LF = 0, NFEAT * D * RROWS               # lf[m]    [128,2048] each, f8
O8_FE, L8_FE = L8_LF, NFEAT * D * FROWS           # feats[m] [128,4096] each, f8
BLOB8 = O8_FE + L8_FE

# rows: [1, x, x2, |x|, x|x|, sign, x3] ; cols: rhs [y, y2, |y|, y|y|]
NEGC = np.array([
    [-2.64634495e-03, 2.57689506e-02, -1.16234565e+00, 2.03689490e-03],
    [2.17274690e+00, -1.19240610e-02, 2.07460839e-02, -7.70343959e-01],
    [-5.45617985e-03, 1.79038107e-01, -4.85291958e-01, 3.84314870e-03],
    [9.64919943e-03, -4.85617042e-01, 1.75258219e+00, -6.89594261e-03],
    [-1.13944638e+00, 1.23156002e-02, -2.10905615e-02, 5.43146372e-01],
    [-3.23009975e-02, 1.92518265e-03, -3.08780512e-03, 9.46847629e-03],
    [1.74482226e-01, -3.03717307e-03, 5.07844985e-03, -9.47937220e-02],
], dtype=np.float32)

_CACHE = {}


def build_nc():
    from contextlib import ExitStack

    import concourse.bass as bass  # noqa: F401
    import concourse.mybir as mybir
    import concourse.tile as tile
    from concourse import bacc, library_config
    from concourse.bass import ts

    dt = mybir.dt
    AX = mybir.AxisListType
    OP = mybir.AluOpType
    AF = mybir.ActivationFunctionType

    nc = bacc.Bacc("TRN2", debug=False)
    blob = nc.dram_tensor("blob", [BLOB], dt.bfloat16, kind="ExternalInput")
    blob8 = nc.dram_tensor("blob8", [BLOB8], dt.float8e4,
                           kind="ExternalInput")
    outm = nc.dram_tensor("outm", [RROWS], dt.float32, kind="ExternalOutput")

    r_ap = blob.ap()[OFF_R:OFF_R + LEN_R].rearrange(
        "(t p d) -> p t d", p=128, d=D)                      # [128, NIT, D]
    f_ap = blob.ap()[OFF_F:OFF_F + LEN_F].rearrange(
        "(t p d) -> p t d", p=128, d=D)                      # [128, NFT, D]
    lf_aps = [blob8.ap()[O8_LF + m * D * RROWS:
                         O8_LF + (m + 1) * D * RROWS].rearrange(
        "(p c) -> p c", p=128) for m in range(NFEAT)]        # [128, RROWS]
    fe_aps = [blob8.ap()[O8_FE + m * D * FROWS:
                         O8_FE + (m + 1) * D * FROWS].rearrange(
        "(p c) -> p c", p=128) for m in range(NFEAT)]        # [128, FROWS]
    v_wrap_ap = blob.ap()[OFF_V:OFF_V + LEN_V].rearrange(
        "(t p) -> p t", p=128)                               # [128, NFT]

    with ExitStack() as ctx:
        tc = ctx.enter_context(tile.TileContext(nc))
        persist = ctx.enter_context(tc.tile_pool(name="persist", bufs=1))

        feats = [persist.tile([128, FROWS], dt.float8e4, tag=f"feat{m}",
                              name=f"feat{m}")
                 for m in range(NFEAT)]
        lf = [persist.tile([128, RROWS], dt.float8e4, tag=f"lf{m}",
                           name=f"lf{m}")
              for m in range(NFEAT)]
        rt_all = persist.tile([128, NIT, D], dt.bfloat16, tag="rt_all")
        mins_all = persist.tile([128, NIT], dt.float32, tag="mins")
        faug = ctx.enter_context(
            tc.tile_pool(name="dramp", bufs=1, space="DRAM")
        ).tile([FROWS, AUGW], dt.bfloat16, tag="faug", name="faug")

        # ---------------- stage A: loads (all features host-built) -----
        with tc.tile_pool(name="stage", bufs=1) as stage:
            for m in range(NFEAT):
                nc.sync.dma_start(feats[m][:], fe_aps[m])
                nc.sync.dma_start(lf[m][:], lf_aps[m])
            nc.sync.dma_start(rt_all[:], r_ap)

            # gather table: fake rows + v appended, written once to DRAM
            fsb = stage.tile([128, NFT, D], dt.bfloat16, tag="fsb")
            nc.scalar.dma_start(fsb[:], f_ap)
            fsa = stage.tile([128, NFT, AUGW], dt.bfloat16, tag="fsa")
            nc.vector.tensor_copy(fsa[:, :, 0:D], fsb[:])
            vsb = stage.tile([128, NFT], dt.bfloat16, tag="vsb")
            nc.scalar.dma_start(vsb[:], v_wrap_ap)
            nc.vector.tensor_copy(fsa[:, :, D], vsb[:])
            nc.scalar.dma_start(
                faug[:].rearrange("(t p) w -> p t w", p=128), fsa[:])

        # ---------------- stage B: proxy + select + exact ----------------
        nc.gpsimd.load_library(library_config.mlp)
        with tc.tile_pool(name="work", bufs=3) as work, \
             tc.tile_pool(name="psum", bufs=8, space="PSUM") as psum, \
             tc.tile_pool(name="drams", bufs=4, space="DRAM") as dpool, \
             tc.tile_pool(name="small", bufs=6) as small:
            def exact(te, fge):
                rt = rt_all[:, te, :]
                diff = work.tile([128, NCAND, D], dt.bfloat16, tag="diff",
                                 name=f"diff{te}")
                nc.vector.tensor_tensor(
                    diff[:], fge[:, :, 0:D],
                    rt[:, None, :].to_broadcast((128, NCAND, D)), OP.subtract)
                d1c = small.tile([128, NCAND], dt.float32, tag="d1c",
                                 name=f"d1c{te}")
                nc.vector.tensor_reduce(d1c[:], diff[:], axis=AX.X, op=OP.add,
                                        apply_absolute_value=True)
                vc = small.tile([128, NCAND], dt.float32, tag="vc",
                                name=f"vc{te}")
                nc.scalar.copy(vc[:], fge[:, :, D])
                gc = small.tile([128, NCAND], dt.float32, tag="gc",
                                name=f"gc{te}")
                nc.vector.tensor_tensor(gc[:], d1c[:], vc[:], OP.subtract)
                nc.vector.tensor_reduce(mins_all[:, te:te + 1], gc[:],
                                        axis=AX.X, op=OP.min)

            pend = []
            for t in range(NIT):
                score = work.tile([128, FROWS], dt.float16, tag="score")
                pss = [psum.tile([128, JT], dt.float32, tag="ps",
                                 name=f"ps{t}_{k}") for k in range(NJT)]
                # two 4-bank sub-groups: copies of group A overlap matmuls
                # of group B so the PE never waits on a full-tile drain
                for jg in range(2):
                    for jj in range(jg * 4, jg * 4 + 4):
                        for m in range(NFEAT):
                            nc.tensor.matmul(
                                pss[jj][:],
                                lf[m][:, ts(t, 128)],
                                feats[m][:, ts(jj, JT)],
                                start=(m == 0), stop=(m == NFEAT - 1))
                    for jj in range(jg * 4, jg * 4 + 4):
                        nc.scalar.copy(score[:, ts(jj, JT)], pss[jj][:])

                mx = small.tile([128, 8], dt.float16, tag="mx")
                nc.vector.max(mx[:], score[:])
                idx = small.tile([128, 8], dt.uint16, tag="idx")
                nc.vector.max_index(idx[:], mx[:], score[:])

                # reshuffle indices to the wrapped dma_gather layout via
                # DRAM; only the top-NCAND of the 8 found indices are used.
                # These stay on the sync queue: on gpsimd they would
                # serialize with the gathers, on scalar with the copies.
                idram = dpool.tile([128 * NCAND], dt.uint16, tag="idram")
                nc.sync.dma_start(idram.rearrange("(p c) -> p c", c=NCAND),
                                  idx[:, 0:NCAND])
                idxw = small.tile([128, 8 * NCAND], dt.uint16, tag="idxw")
                wrap = idram.rearrange("(u tt c) -> tt c u", u=8, tt=16,
                                       c=NCAND)
                for q in range(8):
                    nc.sync.dma_start(
                        idxw[16 * q:16 * (q + 1), :].rearrange(
                            "p (c u) -> p c u", c=NCAND),
                        wrap)

                fg = work.tile([128, NCAND, AUGW], dt.bfloat16, tag="fg")
                nc.gpsimd.dma_gather(
                    fg[:], faug[:], idxw[:].bitcast(dt.int16),
                    num_idxs=NCAND * 128, num_idxs_reg=NCAND * 128,
                    elem_size=AUGW)

                pend.append((t, fg))
                # exact recompute runs two tiles behind selection so the
                # in-order DVE stream never waits on an in-flight gather
                if len(pend) >= 3:
                    exact(*pend.pop(0))

            while pend:
                exact(*pend.pop(0))
            nc.sync.dma_start(outm.ap().rearrange("(t p) -> p t", p=128),
                              mins_all[:])
    nc.compile()
    return nc


def prepare_in_maps(real, fake, v):
    bf = ml_dtypes.bfloat16
    real = np.asarray(real, dtype=np.float32)
    fake = np.asarray(fake, dtype=np.float32)
    v32 = np.asarray(v, dtype=np.float32)
    real_bf = real.astype(bf)
    fake_bf = fake.astype(bf)
    v_bf = v32.astype(bf)

    f8 = ml_dtypes.float8_e4m3
    # lhs feature maps, mixed by NEGC on host in f32:  LF[m] = [NR, D]
    x = real
    ax = np.abs(x)
    basis = np.stack([np.ones_like(x), x, x * x, ax, x * ax, np.sign(x),
                      x * x * x])                       # [7, NR, D]
    LF = np.tensordot(NEGC, basis, axes=(0, 0))         # [4, NR, D] f32
    LF[1, :, 127] = 1.0                                 # sacrifice-row partner
    LFT = LF.astype(f8).transpose(0, 2, 1)              # [4, D, NR]
    # rhs feature maps [y, y^2, |y|, y|y|]; feature 1 row k=127 carries +v
    y = fake
    ay = np.abs(y)
    FE = np.stack([y, y * y, ay, y * ay])               # [4, NF, D] f32
    FE[1, :, 127] = v32
    FET = FE.astype(f8).transpose(0, 2, 1)              # [4, D, NF]

    in_maps = []
    for c in range(NCORES):
        a, b = c // FSH, c % FSH
        rsl = slice(a * RROWS, (a + 1) * RROWS)
        fsl = slice(b * FROWS, (b + 1) * FROWS)
        blobv = np.empty(BLOB, dtype=bf)
        blobv[OFF_R:OFF_R + LEN_R] = real_bf[rsl].ravel()
        blobv[OFF_F:OFF_F + LEN_F] = fake_bf[fsl].ravel()
        blobv[OFF_V:OFF_V + LEN_V] = v_bf[fsl]
        blob8v = np.empty(BLOB8, dtype=f8)
        blob8v[O8_LF:O8_LF + L8_LF] = \
            np.ascontiguousarray(LFT[:, :, rsl]).ravel()
        blob8v[O8_FE:O8_FE + L8_FE] = \
            np.ascontiguousarray(FET[:, :, fsl]).ravel()
        in_maps.append({"blob": blobv, "blob8": blob8v})
    return in_maps


def run(real, fake, v, trace=False):
    from concourse.bass_utils import run_bass_kernel_spmd
    if "nc" not in _CACHE:
        _CACHE["nc"] = build_nc()
    nc = _CACHE["nc"]
    in_maps = prepare_in_maps(real, fake, v)
    try:
        res = run_bass_kernel_spmd(nc, in_maps, core_ids=list(range(NCORES)),
                                   trace=trace)
    except ModuleNotFoundError:
        res = run_bass_kernel_spmd(nc, in_maps, core_ids=list(range(NCORES)),
                                   trace=False)
    except Exception:
        # transient device hiccup (e.g. NRT exec-unit recovery): retry once
        time.sleep(10)
        res = run_bass_kernel_spmd(nc, in_maps, core_ids=list(range(NCORES)),
                                   trace=False)
    mins = np.stack([res.results[c]["outm"] for c in range(NCORES)])
    rowmins = np.minimum(mins[0::FSH], mins[1::FSH])     # [RSH, RROWS]
    vmean = float(np.asarray(v, dtype=np.float32).mean())
    out = np.float32(-vmean - rowmins.mean(dtype=np.float64))
    return out, res


def kernel(real_objects, fake_objects, fake_validity):
    out, _ = run(real_objects, fake_objects, fake_validity)
    return out
